# revision 4
# baseline (speedup 1.0000x reference)
"""Trainium2 Bass kernel for DiagonalPositiveLinear:
    out[b, f] = input[b, f] * exp(log_weight[f])

Full-input contract: kernel() takes the full (8192, 4096) f32 input plus the
(4096,) f32 log_weight, shards rows across 8 NeuronCores (pure data parallel),
runs a raw-Bass kernel per core, and concatenates the row shards back.

Memory-bound: per core 16 MiB in + 16 MiB out over HBM (~358 GB/s/core),
roofline ~94 us. Raw Bass (not Tile) because this toolchain's walrus allows
only ONE sync wait per instruction; all cross-engine sync is standalone
wait_ge instructions and per-tile DMA semaphores.

Pipeline per core (4 tiles of [128 partitions x 8192 f32] = 4 MiB each):
  SP    : 4 tile loads (HWDGE ring A), no waits
  Pool  : broadcast-load log_weight into 128 partitions (SWDGE)
  ACT   : exp(log_weight); then per tile: wait mul done -> store (HWDGE ring B)
  DVE   : wait exp; per tile: wait load -> in-place multiply by exp(w)
"""

import numpy as np

import concourse.bass as bass
from concourse import mybir
from concourse.bass_utils import run_bass_kernel_spmd

N_CORES = 8
ROWS, FEATS = 8192, 4096
SHARD_ROWS = ROWS // N_CORES          # 1024 rows per core
P = 128                               # SBUF partitions
T = 2                                 # rows-per-partition packed along free dim
TILE_FREE = T * FEATS                 # 8192 f32 = 32 KiB per partition
N_TILES = SHARD_ROWS // (P * T)       # 4 tiles of [128, 8192] (4 MiB each)

_F32 = mybir.dt.float32

_cached_nc = None


def _build_bass():
    nc = bass.Bass()
    # Host reshapes each row shard to [N_TILES, P, TILE_FREE]; contiguity makes
    # tile i / partition p hold rows (i*P*T + p*T .. +T) back to back, so the
    # free dim is T full feature rows concatenated.
    x = nc.declare_dram_parameter("x", [N_TILES, P, TILE_FREE], _F32, isOutput=False)
    w = nc.declare_dram_parameter("w", [FEATS], _F32, isOutput=False)
    y = nc.declare_dram_parameter("y", [N_TILES, P, TILE_FREE], _F32, isOutput=True)

    with (
        nc.sbuf_tensor([P, N_TILES * TILE_FREE], _F32) as buf,
        nc.sbuf_tensor([P, FEATS], _F32) as wraw,
        nc.sbuf_tensor([P, FEATS], _F32) as wt,
        nc.semaphore("lw_sem") as lw_sem,      # log_weight broadcast load done
        nc.semaphore("wexp_sem") as wexp_sem,  # exp(w) computed
        nc.semaphore("ld0") as ld0,            # per-tile load completions
        nc.semaphore("ld1") as ld1,
        nc.semaphore("ld2") as ld2,
        nc.semaphore("ld3") as ld3,
        nc.semaphore("mul_sem") as mul_sem,    # per-tile multiply done (in order)
        nc.semaphore("st_sem") as st_sem,      # store completions (total only)
        nc.Block() as block,
    ):
        ld_sems = [ld0, ld1, ld2, ld3]
        assert len(ld_sems) == N_TILES

        def tile_buf(i):
            return buf[:, i * TILE_FREE : (i + 1) * TILE_FREE]

        @block.sync
        def _(sync):
            for i in range(N_TILES):
                sync.dma_start(out=tile_buf(i), in_=x[i]).then_inc(ld_sems[i], 16)

        @block.gpsimd
        def _(gpsimd):
            gpsimd.dma_start(
                out=wraw[:], in_=w[None, :].to_broadcast((P, FEATS))
            ).then_inc(lw_sem, 16)

        @block.vector
        def _(vector):
            vector.wait_ge(wexp_sem, 1)
            for i in range(N_TILES):
                vector.wait_ge(ld_sems[i], 16)
                t = tile_buf(i)
                for j in range(T):
                    sl = t[:, j * FEATS : (j + 1) * FEATS]
                    ins = vector.tensor_mul(sl, sl, wt[:])
                ins.then_inc(mul_sem, 1)

        @block.scalar
        def _(scalar):
            scalar.wait_ge(lw_sem, 16)
            scalar.activation(
                wt[:], wraw[:], mybir.ActivationFunctionType.Exp
            ).then_inc(wexp_sem, 1)
            for i in range(N_TILES):
                scalar.wait_ge(mul_sem, i + 1)
                scalar.dma_start(out=y[i], in_=tile_buf(i)).then_inc(st_sem, 16)
            scalar.wait_ge(st_sem, 16 * N_TILES)

    return nc


def _get_nc():
    global _cached_nc
    if _cached_nc is None:
        _cached_nc = _build_bass()
    return _cached_nc


def _run(input, log_weight, trace=False, **spmd_kwargs):
    input = np.ascontiguousarray(np.asarray(input, dtype=np.float32))
    log_weight = np.ascontiguousarray(np.asarray(log_weight, dtype=np.float32))
    nc = _get_nc()
    in_maps = []
    for c in range(N_CORES):
        shard = input[c * SHARD_ROWS : (c + 1) * SHARD_ROWS]
        in_maps.append(
            {
                "x": shard.reshape(N_TILES, P, TILE_FREE),
                "w": log_weight,
            }
        )
    res = run_bass_kernel_spmd(
        nc, in_maps, core_ids=list(range(N_CORES)), trace=trace, **spmd_kwargs
    )
    out = np.concatenate(
        [r["y"].reshape(SHARD_ROWS, FEATS) for r in res.results], axis=0
    )
    return out, res


def kernel(input, log_weight):
    out, _ = _run(input, log_weight, trace=False)
    return out


# revision 5
# speedup vs baseline: 53.2542x; 53.2542x over previous
"""Trainium2 Bass kernel for DiagonalPositiveLinear:
    out[b, f] = input[b, f] * exp(log_weight[f])

Full-input contract: kernel() takes the full (8192, 4096) f32 input plus the
(4096,) f32 log_weight, shards rows across 8 NeuronCores (pure data parallel),
runs a raw-Bass kernel per core, and concatenates the row shards back.

Memory-bound: per core 16 MiB in + 16 MiB out over HBM (~358 GB/s/core),
roofline ~94 us. Raw Bass (not Tile) because this toolchain's walrus allows
only ONE sync wait per instruction; all cross-engine sync is standalone
wait_ge instructions and per-tile DMA semaphores.

Pipeline per core (4 tiles of [128 partitions x 8192 f32] = 4 MiB each):
  SP    : 4 tile loads (HWDGE ring A), no waits
  Pool  : broadcast-load log_weight into 128 partitions (SWDGE)
  ACT   : exp(log_weight); then per tile: wait mul done -> store (HWDGE ring B)
  DVE   : wait exp; per tile: wait load -> in-place multiply by exp(w)
"""

import numpy as np

import concourse.bass as bass
from concourse import mybir
from concourse.bass_utils import run_bass_kernel_spmd

N_CORES = 8
ROWS, FEATS = 8192, 4096
SHARD_ROWS = ROWS // N_CORES          # 1024 rows per core
P = 128                               # SBUF partitions
T = 2                                 # rows-per-partition packed along free dim
TILE_FREE = T * FEATS                 # 8192 f32 = 32 KiB per partition
N_TILES = SHARD_ROWS // (P * T)       # 4 tiles of [128, 8192] (4 MiB each)

_F32 = mybir.dt.float32

_cached_nc = None


def _build_bass(repeats=1):
    """repeats>1 builds a timing variant: the full load/mul/store pipeline is
    executed `repeats` times over the same data, so steady-state kernel time
    can be extracted as the slope w.r.t. repeats (amortizes dispatch cost)."""
    nc = bass.Bass()
    # Host reshapes each row shard to [N_TILES, P, TILE_FREE]; contiguity makes
    # tile i / partition p hold rows (i*P*T + p*T .. +T) back to back, so the
    # free dim is T full feature rows concatenated.
    x = nc.declare_dram_parameter("x", [N_TILES, P, TILE_FREE], _F32, isOutput=False)
    w = nc.declare_dram_parameter("w", [FEATS], _F32, isOutput=False)
    y = nc.declare_dram_parameter("y", [N_TILES, P, TILE_FREE], _F32, isOutput=True)

    with (
        nc.sbuf_tensor([P, N_TILES * TILE_FREE], _F32) as buf,
        nc.sbuf_tensor([P, FEATS], _F32) as wraw,
        nc.sbuf_tensor([P, FEATS], _F32) as wt,
        nc.semaphore("lw_sem") as lw_sem,      # log_weight broadcast load done
        nc.semaphore("wexp_sem") as wexp_sem,  # exp(w) computed
        nc.semaphore("ld0") as ld0,            # per-tile load completions
        nc.semaphore("ld1") as ld1,
        nc.semaphore("ld2") as ld2,
        nc.semaphore("ld3") as ld3,
        nc.semaphore("mul_sem") as mul_sem,    # per-tile multiply done (in order)
        nc.semaphore("st_sem") as st_sem,      # store completions (total only)
        nc.Block() as block,
    ):
        ld_sems = [ld0, ld1, ld2, ld3]
        assert len(ld_sems) == N_TILES

        def tile_buf(i):
            return buf[:, i * TILE_FREE : (i + 1) * TILE_FREE]

        @block.sync
        def _(sync):
            for r in range(repeats):
                if r > 0:
                    # buffer slots reused across repeats: all repeat r-1
                    # stores must have drained (total-count semantics)
                    sync.wait_ge(st_sem, 16 * N_TILES * r)
                for i in range(N_TILES):
                    sync.dma_start(out=tile_buf(i), in_=x[i]).then_inc(
                        ld_sems[i], 16
                    )

        @block.gpsimd
        def _(gpsimd):
            gpsimd.dma_start(
                out=wraw[:], in_=w[None, :].to_broadcast((P, FEATS))
            ).then_inc(lw_sem, 16)

        @block.vector
        def _(vector):
            vector.wait_ge(wexp_sem, 1)
            for r in range(repeats):
                for i in range(N_TILES):
                    vector.wait_ge(ld_sems[i], 16 * (r + 1))
                    t = tile_buf(i)
                    for j in range(T):
                        sl = t[:, j * FEATS : (j + 1) * FEATS]
                        ins = vector.tensor_mul(sl, sl, wt[:])
                    ins.then_inc(mul_sem, 1)

        @block.scalar
        def _(scalar):
            scalar.wait_ge(lw_sem, 16)
            scalar.activation(
                wt[:], wraw[:], mybir.ActivationFunctionType.Exp
            ).then_inc(wexp_sem, 1)
            for r in range(repeats):
                for i in range(N_TILES):
                    scalar.wait_ge(mul_sem, N_TILES * r + i + 1)
                    scalar.dma_start(out=y[i], in_=tile_buf(i)).then_inc(st_sem, 16)
            scalar.wait_ge(st_sem, 16 * N_TILES * repeats)

    return nc


def _get_nc():
    global _cached_nc
    if _cached_nc is None:
        _cached_nc = _build_bass()
    return _cached_nc


def _run(input, log_weight, trace=False, **spmd_kwargs):
    input = np.ascontiguousarray(np.asarray(input, dtype=np.float32))
    log_weight = np.ascontiguousarray(np.asarray(log_weight, dtype=np.float32))
    nc = _get_nc()
    in_maps = []
    for c in range(N_CORES):
        shard = input[c * SHARD_ROWS : (c + 1) * SHARD_ROWS]
        in_maps.append(
            {
                "x": shard.reshape(N_TILES, P, TILE_FREE),
                "w": log_weight,
            }
        )
    res = run_bass_kernel_spmd(
        nc, in_maps, core_ids=list(range(N_CORES)), trace=trace, **spmd_kwargs
    )
    out = np.concatenate(
        [r["y"].reshape(SHARD_ROWS, FEATS) for r in res.results], axis=0
    )
    return out, res


def kernel(input, log_weight):
    out, _ = _run(input, log_weight, trace=False)
    return out
